# revision 15
# baseline (speedup 1.0000x reference)
"""Trainium2 Bass kernel for nn_FeatureMapTemporalModule.

GAT-style temporal attention over B*N independent 64-node temporal graphs.
Sharding: data-parallel over the B*N=784 graph axis, 98 graphs per core
(core m handles b = m//2, spatial locations n in [98*(m%2), 98*(m%2)+98)).
Params (Wg-derived matrices, W2, LN) are replicated.

Host prep: per-core x slice transposed to [C, N, T] so every device DMA is
contiguous; output comes back [C, N, T] per core and is re-assembled on host.
All floating-point math (norms, top-k, matmuls, softmax, ...) runs on device.
"""
import os
import numpy as np

import concourse.bacc as bacc
import concourse.bass as bass
import concourse.tile as tile
from concourse import mybir
from concourse.bass_utils import run_bass_kernel_spmd

f32 = mybir.dt.float32
f32r = mybir.dt.float32r
AF = mybir.ActivationFunctionType
ALU = mybir.AluOpType
AX = mybir.AxisListType

GAMMA = 0.6
BIAS = 0.2
ALPHA = 0.2
MASK_RATE = 0.3
NH = 8
C = 512
DH = C // NH
B, T, H, W = 4, 64, 14, 14
N = H * W
CORES = 8
G_CORE = (B * N) // CORES          # 98 graphs per core
NPAIR = G_CORE // 2                # 49
K_TOP = int(MASK_RATE * T)         # 19
NCHUNK = C // 128                  # 4

USE_F32R = os.environ.get("KBASS_F32R", "0") == "1"


def _r(ap):
    return ap.bitcast(f32r) if USE_F32R else ap


def build_program(nc):
    xb = nc.dram_tensor("xb", [C, G_CORE, T], f32, kind="ExternalInput")
    what = nc.dram_tensor("what", [C, C], f32, kind="ExternalInput")
    w12 = nc.dram_tensor("w12", [C, 2 * NH], f32, kind="ExternalInput")
    w2 = nc.dram_tensor("w2", [C, C], f32, kind="ExternalInput")
    b2g = nc.dram_tensor("b2g", [C, 1], f32, kind="ExternalInput")
    lng = nc.dram_tensor("lng", [C, 1], f32, kind="ExternalInput")
    lnb = nc.dram_tensor("lnb", [C, 1], f32, kind="ExternalInput")
    ydev = nc.dram_tensor("ydev", [C, G_CORE, T], f32, kind="ExternalOutput")
    seld = nc.dram_tensor("seld", [NPAIR, 128], f32)
    ftd = nc.dram_tensor("ftd", [NPAIR, 2 * NH, 128], f32)
    f12d = nc.dram_tensor("f12d", [NPAIR, 128, 2 * NH], f32)

    # stage-2 groups of 4 graphs (2 pairs); x-load groups of 8 graphs
    XG = 8
    ngx = (G_CORE + XG - 1) // XG   # 13 (12 full + one of 2)

    with tile.TileContext(nc) as tc:
        import contextlib
        ctx = contextlib.ExitStack()
        with ctx:
            singles = ctx.enter_context(tc.tile_pool(name="singles", bufs=1))
            xpool = ctx.enter_context(tc.tile_pool(name="xpool", bufs=2))
            ew = ctx.enter_context(tc.tile_pool(name="ew", bufs=3))
            cew = ctx.enter_context(tc.tile_pool(name="cew", bufs=2))
            tts = ctx.enter_context(tc.tile_pool(name="tts", bufs=3))
            pfx = ctx.enter_context(tc.tile_pool(name="pfx", bufs=6))
            pbig = ctx.enter_context(tc.tile_pool(name="pbig", bufs=4, space="PSUM"))
            psm = ctx.enter_context(tc.tile_pool(name="psm", bufs=2, space="PSUM"))
            pw2 = ctx.enter_context(tc.tile_pool(name="pw2", bufs=2, space="PSUM"))

            # ---------------- constants ----------------
            Wsb = []
            W2sb = []
            w12sb = []
            for k in range(NCHUNK):
                wt = singles.tile([128, C], f32, tag=f"wsb{k}")
                nc.sync.dma_start(out=wt, in_=what[k * 128:(k + 1) * 128, :])
                Wsb.append(wt)
                w2t = singles.tile([128, C], f32, tag=f"w2sb{k}")
                nc.sync.dma_start(out=w2t, in_=w2[k * 128:(k + 1) * 128, :])
                W2sb.append(w2t)
                wft = singles.tile([128, 2 * NH], f32, tag=f"w12sb{k}")
                nc.sync.dma_start(out=wft, in_=w12[k * 128:(k + 1) * 128, :])
                w12sb.append(wft)
            b2c = singles.tile([128, NCHUNK], f32, tag="b2c")
            lngc = singles.tile([128, NCHUNK], f32, tag="lngc")
            lnbc = singles.tile([128, NCHUNK], f32, tag="lnbc")
            for k in range(NCHUNK):
                nc.sync.dma_start(out=b2c[:, k:k + 1], in_=b2g[k * 128:(k + 1) * 128, :])
                nc.sync.dma_start(out=lngc[:, k:k + 1], in_=lng[k * 128:(k + 1) * 128, :])
                nc.sync.dma_start(out=lnbc[:, k:k + 1], in_=lnb[k * 128:(k + 1) * 128, :])

            ones128 = singles.tile([128, 128], f32, tag="ones128")
            nc.vector.memset(ones128, 1.0)
            ident = singles.tile([128, 128], f32, tag="ident")
            nc.gpsimd.affine_select(out=ident, in_=ones128, pattern=[[1, 128]],
                                    base=0, channel_multiplier=-1,
                                    compare_op=ALU.is_equal, fill=0.0)
            e2col = singles.tile([128, 2], f32, tag="e2col")
            nc.vector.memset(e2col, 0.0)
            nc.vector.memset(e2col[0:64, 0:1], 1.0)
            nc.vector.memset(e2col[64:128, 1:2], 1.0)
            onecol = singles.tile([128, 1], f32, tag="onecol")
            nc.vector.memset(onecol, 1.0)
            epsrow = singles.tile([1, 1], f32, tag="epsrow")
            nc.vector.memset(epsrow, 1e-5)

            MAGS2 = singles.tile([128, NPAIR], f32, tag="mags2")
            sel = singles.tile([NPAIR, 128], f32, tag="sel")

            # persistent block-diag attention tiles (2 ping-pong sets x 8 heads)
            abd = []
            for s in range(3):
                row = []
                for n in range(NH):
                    t_ = singles.tile([128, 128], f32, tag=f"abd{s}_{n}")
                    nc.vector.memset(t_, 0.0)
                    row.append(t_)
                abd.append(row)

            def load_xg(gi):
                g0 = gi * XG
                gn = min(XG, G_CORE - g0)
                tiles = []
                for k in range(NCHUNK):
                    xt = xpool.tile([128, XG, T], f32, tag=f"xg{k}")
                    nc.sync.dma_start(
                        out=xt[:, :gn, :],
                        in_=xb[k * 128:(k + 1) * 128, g0:g0 + gn, :])
                    tiles.append(xt)
                return tiles, gn

            def pair_lhsT(xt, lp):
                # [c, (g2, t)] strided view: M = g*64 + t
                apv = bass.AP(tensor=xt.tensor, offset=xt.offset, ap=list(xt.ap))
                base = apv[:, 2 * lp:2 * lp + 2, :]
                return bass.AP(tensor=base.tensor, offset=base.offset,
                               ap=[base.ap[0], [T, 2], [1, T]])

            # ================= sweep 1: mags + topk =================
            for gi in range(ngx):
                xts, gn = load_xg(gi)
                npg = gn // 2
                for lp in range(npg):
                    p = gi * (XG // 2) + lp
                    q = (lp // 2) * 4
                    qn = min(4, gn - q)
                    gram = pbig.tile([128, qn * T], f32, tag="pbig")
                    fps = psm.tile([128, 2 * NH], f32, tag="psm")
                    for k in range(NCHUNK):
                        lhsT = pair_lhsT(xts[k], lp)
                        rhs = bass.AP(tensor=xts[k].tensor, offset=xts[k].offset,
                                      ap=list(xts[k].ap))[:, q:q + qn, :]
                        nc.tensor.matmul(gram, _r(lhsT), _r(rhs),
                                         start=(k == 0), stop=(k == NCHUNK - 1))
                        nc.tensor.matmul(fps, _r(lhsT), _r(w12sb[k]),
                                         start=(k == 0), stop=(k == NCHUNK - 1))
                    off = (lp % 2) * 128 if qn == 4 else 0
                    dg = tts.tile([128, 128], f32, tag="dg")
                    nc.vector.tensor_mul(dg, ident, gram[:, off:off + 128])
                    nc.vector.reduce_sum(MAGS2[:, p:p + 1], dg, axis=AX.X)
                    f12s1 = tts.tile([128, 2 * NH], f32, tag="f12s1")
                    nc.any.tensor_copy(f12s1, fps)
                    nc.sync.dma_start(out=f12d[p], in_=f12s1)
                    f12t_ps = psm.tile([2 * NH, 128], f32, tag="psm")
                    nc.tensor.transpose(_r(f12t_ps), _r(f12s1), _r(ident))
                    f12t1 = tts.tile([2 * NH, 128], f32, tag="f12t1")
                    nc.any.tensor_copy(f12t1, f12t_ps)
                    nc.sync.dma_start(out=ftd[p], in_=f12t1)

            # transpose MAGS2 -> [49, 128], topk
            mt_ps = pbig.tile([NPAIR, 128], f32, tag="pbig")
            nc.tensor.transpose(_r(mt_ps), _r(MAGS2), _r(ident))
            MG = singles.tile([NPAIR, 2, T], f32, tag="mgw")
            nc.any.tensor_copy(MG, mt_ps.rearrange("p (a b) -> p a b", a=2))
            nc.vector.memset(sel, 0.0)
            selv = sel.rearrange("p (a b) -> p a b", a=2)
            mrow = singles.tile([NPAIR, 2], f32, tag="mrow")
            ismax = singles.tile([NPAIR, T], f32, tag="ismax")
            for it in range(K_TOP):
                nc.vector.reduce_max(mrow, MG, axis=AX.X)
                for g in range(2):
                    nc.vector.tensor_scalar(out=ismax, in0=MG[:, g, :],
                                            scalar1=mrow[:, g:g + 1], scalar2=None,
                                            op0=ALU.is_ge)
                    nc.vector.tensor_tensor(out=selv[:, g, :], in0=selv[:, g, :],
                                            in1=ismax, op=ALU.max)
                    nc.vector.scalar_tensor_tensor(out=MG[:, g, :], in0=ismax,
                                                   scalar=-1e30, in1=MG[:, g, :],
                                                   op0=ALU.mult, op1=ALU.add)

            nc.sync.dma_start(out=seld[:, :], in_=sel)

            # ================= sweep 2: main pipeline =================
            for gi in range(ngx):
                xts, gn = load_xg(gi)
                npg = gn // 2
                nq = (gn + 3) // 4
                for qq in range(nq):
                    qgn = min(4, gn - qq * 4)
                    qpairs = qgn // 2
                    tmpT = []
                    for k in range(NCHUNK):
                        tmpT_k = cew.tile([128, qgn * T], f32, tag=f"tmpT{k}")
                        tmpT.append(tmpT_k)
                    for lpq in range(qpairs):
                        lp = qq * 2 + lpq
                        p = gi * (XG // 2) + lp
                        # ---- stage 1: h + f12 ----
                        hps = pbig.tile([128, C], f32, tag="pbig")
                        for k in range(NCHUNK):
                            lhsT = pair_lhsT(xts[k], lp)
                            nc.tensor.matmul(hps, _r(lhsT), _r(Wsb[k]),
                                             start=(k == 0), stop=(k == NCHUNK - 1))
                        hsb = ew.tile([128, C], f32, tag="hsb")
                        nc.scalar.copy(hsb, hps)
                        f12 = pfx.tile([128, 2 * NH], f32, tag="f12")
                        nc.sync.dma_start(out=f12, in_=f12d[p])

                        # ---- masks ----
                        selcol = pfx.tile([128, 1], f32, tag="selcol")
                        nc.sync.dma_start(out=selcol,
                                          in_=seld[p:p + 1, :].rearrange("a b -> b a"))
                        selrow = pfx.tile([128, T], f32, tag="selrow")
                        for g in range(2):
                            nc.sync.dma_start(
                                out=selrow[g * 64:(g + 1) * 64, :],
                                in_=seld[p:p + 1, g * 64:(g + 1) * 64].to_broadcast([64, 64]))
                        m01 = tts.tile([128, T], f32, tag="m01")
                        nc.vector.tensor_scalar(out=m01, in0=selrow, scalar1=selcol,
                                                scalar2=None, op0=ALU.max)

                        # ---- E = f1 + f2, leaky relu, exp, mask ----
                        f2s = pfx.tile([128, C], f32, tag="f2s")
                        for g in range(2):
                            fsrc = ftd[p, NH:2 * NH, g * 64:(g + 1) * 64]
                            fb = bass.AP(tensor=fsrc.tensor, offset=fsrc.offset,
                                         ap=[[0, 64]] + list(fsrc.ap))
                            nc.sync.dma_start(
                                out=f2s[g * 64:(g + 1) * 64, :].rearrange(
                                    "p (n s) -> p n s", n=NH),
                                in_=fb)
                        ee = ew.tile([128, C], f32, tag="ee")
                        for n in range(NH):
                            nc.scalar.activation(ee[:, n * 64:(n + 1) * 64],
                                                 f2s[:, n * 64:(n + 1) * 64],
                                                 AF.Identity, bias=f12[:, n:n + 1],
                                                 scale=1.0)
                        nc.vector.scalar_tensor_tensor(out=ee, in0=ee, scalar=ALPHA,
                                                       in1=ee, op0=ALU.mult, op1=ALU.max)
                        pp = ee
                        nc.scalar.activation(pp, pp, AF.Exp)
                        dd = tts.tile([128, NH], f32, tag="dd")
                        aset = abd[p % 3]
                        for n in range(NH):
                            for g in range(2):
                                sl = slice(g * 64, (g + 1) * 64)
                                nc.vector.scalar_tensor_tensor(
                                    out=aset[n][sl, sl],
                                    in0=pp[sl, n * 64:(n + 1) * 64], scalar=1.0,
                                    in1=m01[sl, :], op0=ALU.mult, op1=ALU.mult,
                                    accum_out=dd[sl, n:n + 1])
                        drec = tts.tile([128, NH], f32, tag="drec")
                        nc.vector.reciprocal(drec, dd)

                        # ---- att @ h ----
                        aops = pbig.tile([128, C], f32, tag="pbig")
                        for n in range(NH):
                            atp = pbig.tile([128, 128], f32, tag="pbig")
                            nc.tensor.transpose(_r(atp), _r(aset[n]), _r(ident))
                            ats = tts.tile([128, 128], f32, tag="ats")
                            nc.any.tensor_copy(ats, atp)
                            nc.tensor.matmul(aops[:, n * 64:(n + 1) * 64], _r(ats),
                                             _r(hsb[:, n * 64:(n + 1) * 64]),
                                             start=True, stop=True)
                        tmp = ew.tile([128, C], f32, tag="tmp")
                        for n in range(NH):
                            nc.vector.tensor_scalar(
                                out=tmp[:, n * 64:(n + 1) * 64],
                                in0=aops[:, n * 64:(n + 1) * 64],
                                scalar1=drec[:, n:n + 1], scalar2=None, op0=ALU.mult)

                        # ---- elu, signed sqrt ----
                        qq_ = ew.tile([128, C], f32, tag="qq")
                        nc.vector.tensor_scalar(out=qq_, in0=tmp, scalar1=0.0,
                                                scalar2=None, op0=ALU.min)
                        nc.scalar.activation(qq_, qq_, AF.Exp)
                        nc.vector.scalar_tensor_tensor(out=qq_, in0=qq_, scalar=-1.0,
                                                       in1=tmp, op0=ALU.add, op1=ALU.max)
                        ab_ = ew.tile([128, C], f32, tag="ab")
                        nc.scalar.activation(ab_, qq_, AF.Abs)
                        sq = ew.tile([128, C], f32, tag="sq")
                        nc.scalar.activation(sq, ab_, AF.Sqrt)
                        sgn = ew.tile([128, C], f32, tag="sgn")
                        nc.scalar.activation(sgn, qq_, AF.Sign)
                        ss = sq
                        nc.vector.tensor_mul(ss, sq, sgn)

                        # ---- normalize over t ----
                        nrm2 = psm.tile([2, C], f32, tag="psm")
                        nc.tensor.matmul(nrm2, _r(e2col), _r(ab_), start=True, stop=True)
                        sr = tts.tile([2, C], f32, tag="sr")
                        nc.scalar.activation(sr, nrm2, AF.Sqrt)
                        nc.vector.tensor_scalar(out=sr, in0=sr, scalar1=1e-12,
                                                scalar2=None, op0=ALU.max)
                        rs = sr
                        nc.vector.reciprocal(rs, rs)

                        # ---- transpose ss -> tmpT with scale ----
                        for k in range(NCHUNK):
                            rstp = psm.tile([128, 2], f32, tag="psm")
                            nc.tensor.transpose(_r(rstp), _r(rs[:, k * 128:(k + 1) * 128]),
                                                _r(ident[0:2, 0:2]))
                            rst = tts.tile([128, 2], f32, tag="rst")
                            nc.any.tensor_copy(rst, rstp)
                            sstp = pbig.tile([128, 128], f32, tag="pbig")
                            nc.tensor.transpose(_r(sstp), _r(ss[:, k * 128:(k + 1) * 128]),
                                                _r(ident))
                            for g in range(2):
                                nc.vector.tensor_scalar(
                                    out=tmpT[k][:, (2 * lpq + g) * 64:(2 * lpq + g + 1) * 64],
                                    in0=sstp[:, g * 64:(g + 1) * 64],
                                    scalar1=rst[:, g:g + 1], scalar2=None, op0=ALU.mult)

                    # ---- stage 2 per quad: W2 + residual + LN ----
                    nq64 = qgn * T
                    wps = []
                    for _wi in range(2):
                        wps_t = pw2.tile([128, 2, nq64], f32, tag="pw2")
                        wps.append(wps_t)
                    for cc in range(NCHUNK):
                        for k in range(NCHUNK):
                            nc.tensor.matmul(wps[cc // 2][:, cc % 2, :],
                                             _r(W2sb[k][:, cc * 128:(cc + 1) * 128]),
                                             _r(tmpT[k]),
                                             start=(k == 0), stop=(k == NCHUNK - 1))
                    sums = psm.tile([1, nq64], f32, tag="psm")
                    sumq = psm.tile([1, nq64], f32, tag="psm")
                    yts = []
                    for cc in range(NCHUNK):
                        yt = cew.tile([128, nq64], f32, tag=f"yt{cc}")
                        xres = bass.AP(tensor=xts[cc].tensor, offset=xts[cc].offset,
                                       ap=list(xts[cc].ap))[:, qq * 4:qq * 4 + qgn, :]
                        nc.vector.scalar_tensor_tensor(out=yt, in0=wps[cc // 2][:, cc % 2, :],
                                                       scalar=b2c[:, cc:cc + 1], in1=xres,
                                                       op0=ALU.add, op1=ALU.add)
                        yts.append(yt)
                        sqt = cew.tile([128, nq64], f32, tag="sqt")
                        nc.vector.tensor_mul(sqt, yt, yt)
                        nc.tensor.matmul(sums, _r(onecol), _r(yt),
                                         start=(cc == 0), stop=(cc == NCHUNK - 1))
                        nc.tensor.matmul(sumq, _r(onecol), _r(sqt),
                                         start=(cc == 0), stop=(cc == NCHUNK - 1))
                    mu = tts.tile([1, nq64], f32, tag="mu")
                    nc.vector.tensor_scalar(out=mu, in0=sums, scalar1=1.0 / C,
                                            scalar2=None, op0=ALU.mult)
                    msq = tts.tile([1, nq64], f32, tag="msq")
                    nc.vector.tensor_mul(msq, mu, mu)
                    varb = tts.tile([1, nq64], f32, tag="varb")
                    nc.vector.scalar_tensor_tensor(out=varb, in0=sumq, scalar=1.0 / C,
                                                   in1=msq, op0=ALU.mult, op1=ALU.subtract)
                    vstd = tts.tile([1, nq64], f32, tag="vstd")
                    nc.scalar.activation(vstd, varb, AF.Sqrt, bias=epsrow[:, 0:1], scale=1.0)
                    rstd = tts.tile([1, nq64], f32, tag="rstd")
                    nc.vector.reciprocal(rstd, vstd)
                    m2 = tts.tile([1, nq64], f32, tag="m2")
                    nc.vector.tensor_mul(m2, mu, rstd)
                    rstdb = cew.tile([128, nq64], f32, tag="rstdb")
                    nc.gpsimd.partition_broadcast(rstdb, rstd, channels=128)
                    m2b = cew.tile([128, nq64], f32, tag="m2b")
                    nc.gpsimd.partition_broadcast(m2b, m2, channels=128)
                    for cc in range(NCHUNK):
                        yln = yts[cc]
                        nc.vector.tensor_mul(yln, yln, rstdb)
                        nc.vector.tensor_sub(yln, yln, m2b)
                        nc.vector.tensor_scalar(out=yln, in0=yln,
                                                scalar1=lngc[:, cc:cc + 1],
                                                scalar2=lnbc[:, cc:cc + 1],
                                                op0=ALU.mult, op1=ALU.add)
                        g0 = gi * XG + qq * 4
                        nc.sync.dma_start(
                            out=ydev[cc * 128:(cc + 1) * 128, g0:g0 + qgn, :],
                            in_=yln.rearrange("p (a b) -> p a b", a=qgn))
    nc.finalize()
    return nc


_CACHE = {}


def _get_program():
    if "nc" not in _CACHE:
        nc = bacc.Bacc("TRN2", target_bir_lowering=False, debug=False,
                       num_devices=CORES)
        _CACHE["nc"] = build_program(nc)
    return _CACHE["nc"]


def kernel(x, Wg, a1, a2, W2, b2, ln_g, ln_b):
    x = np.asarray(x, dtype=np.float32)
    Wg = np.asarray(Wg, dtype=np.float32)
    a1 = np.asarray(a1, dtype=np.float32)
    a2 = np.asarray(a2, dtype=np.float32)
    W2 = np.asarray(W2, dtype=np.float32)
    b2 = np.asarray(b2, dtype=np.float32)
    ln_g = np.asarray(ln_g, dtype=np.float32)
    ln_b = np.asarray(ln_b, dtype=np.float32)

    # host param prep (tiny)
    what = np.ascontiguousarray(Wg.transpose(1, 0, 2).reshape(C, C))  # [c, (n d)]
    w1 = np.einsum("ncd,nd->cn", Wg, a1)
    w2c = np.einsum("ncd,nd->cn", Wg, a2)
    w12 = np.ascontiguousarray(np.concatenate([w1, w2c], axis=1))     # [c, 16]

    xr = x.reshape(B, T, C, N)
    in_maps = []
    for m in range(CORES):
        b = m // 2
        n0 = (m % 2) * G_CORE
        # [T, C, 98] -> [C, 98, T]
        xbm = np.ascontiguousarray(xr[b, :, :, n0:n0 + G_CORE].transpose(1, 2, 0))
        in_maps.append(dict(
            xb=xbm, what=what, w12=w12, w2=W2,
            b2g=b2.reshape(C, 1), lng=ln_g.reshape(C, 1), lnb=ln_b.reshape(C, 1)))

    nc = _get_program()
    res = run_bass_kernel_spmd(nc, in_maps, list(range(CORES)))

    y = np.empty((B, T, C, N), dtype=np.float32)
    for m in range(CORES):
        b = m // 2
        n0 = (m % 2) * G_CORE
        # [C, 98, T] -> [T, C, 98]
        y[b, :, :, n0:n0 + G_CORE] = res.results[m]["ydev"].transpose(2, 0, 1)
    return y.reshape(B, T, C, H, W)
